# revision 30
# baseline (speedup 1.0000x reference)
"""Trainium2 Bass kernel for nn_MoEElementFusion (moe_routing).

Strategy (8 NeuronCores, SPMD):
  Phase 1 (token-data-parallel): each core takes 1/8 of the 8192 (view,token)
  columns and computes, in true fp32 (PE 4-pass) to keep top-k selection
  faithful to the fp32 reference:
      h  = x @ proj_w + proj_b            (feature-major, weights stationary)
      r  = h @ router_w
      d2 = |r|^2 - 2 r.keys^T + |keys|^2  (rk/rr/kk accumulated on PE)
  Outputs h^T and d2^T.

  Host: logits = -sqrt(max(d2,0)), exact top-4 + softmax gates in numpy
  (matches the jax fp32 reference), then builds a balanced slot plan:
  each expert's selected tokens are cut into slots of L=512 columns; the
  8 cores each process S identical-length slots (expert weights + gathered
  h columns + gate rows are per-core input data, so the hot/cold expert
  imbalance is erased no matter how skewed the routing is).

  Phase 2 (compiled at runtime once S is known): per slot, FFN in fp32r
  (full PE rate at N=512):  out^T = (w2^T-mm(gelu(w1^T-mm(h^T) + b1)) + b2) * g
  feature-major throughout; gates applied via a pre-broadcast [128, C] row.

  Host combine: fused[:, tok] += out columns per slot; sum the two views.
"""

import math
import os

import numpy as np

import concourse.bass as bass
import concourse.bacc as bacc
import concourse.mybir as mybir
import concourse.tile as tile
from concourse.bass_utils import run_bass_kernel_spmd

# Problem dims (hardcoded per spec)
V, B, T, D, E, K = 2, 4, 1024, 512, 16, 4
H = 4 * D
N = B * T          # tokens per view
NT = V * N         # total (view, token) columns = 8192
NC = 8             # cores
PC = NT // NC      # phase-1 columns per core = 1024
L = 512            # phase-2 slot length (columns)

F32 = mybir.dt.float32
F32R = mybir.dt.float32r
AF = mybir.ActivationFunctionType
ALU = mybir.AluOpType

DK = D // 128      # 4 k-tiles over D
HK = H // 128      # 16 k-tiles over H

# Phase-2 FFN in fp16 (1 cycle/row on the PE vs 2 for fp32; psum stays fp32).
P2_F16 = os.environ.get("KP2F32") != "1"
# Phase-1 proj/router in fp16; borderline top-4 selections (logit gap below
# REPAIR_MARGIN) are recomputed exactly on host in fp32.
P1_F16 = os.environ.get("KP1F32") != "1"
REPAIR_MARGIN = 0.02

# Filled by kernel() for test harness introspection.
last_stats: dict = {}


def _phase1_nc() -> bass.Bass:
    DT1 = mybir.dt.float16 if P1_F16 else F32
    nc = bacc.Bacc("TRN2", target_bir_lowering=False, num_devices=NC)
    xT = nc.dram_tensor("xT", [D, PC], DT1, kind="ExternalInput")
    pw = nc.dram_tensor("pw", [D, D], DT1, kind="ExternalInput")
    pb = nc.dram_tensor("pb", [128, DK], F32, kind="ExternalInput")
    rw = nc.dram_tensor("rw", [D, D], DT1, kind="ExternalInput")
    kT2 = nc.dram_tensor("kT2", [D, E], DT1, kind="ExternalInput")
    kk1 = nc.dram_tensor("kk1", [1, E], F32, kind="ExternalInput")
    onc = nc.dram_tensor("onc", [128, 1], DT1, kind="ExternalInput")
    onr = nc.dram_tensor("onr", [1, 512], F32, kind="ExternalInput")
    hT = nc.dram_tensor("hT", [D, PC], DT1, kind="ExternalOutput")
    d2T = nc.dram_tensor("d2T", [E, PC], F32, kind="ExternalOutput")

    NCH = PC // 512  # 512-column chunks

    with tile.TileContext(nc) as tc:
        with (
            tc.tile_pool(name="const", bufs=1) as cpool,
            tc.tile_pool(name="act", bufs=1) as apool,
            tc.tile_pool(name="ps", bufs=2, space="PSUM") as pspool,
            tc.tile_pool(name="ps_small", bufs=2, space="PSUM") as psmall,
        ):
            xT_sb = cpool.tile([128, DK, PC], DT1, tag="xT")
            pw_sb = cpool.tile([128, DK, D], DT1, tag="pw")
            rw_sb = cpool.tile([128, DK, D], DT1, tag="rw")
            for k in range(DK):
                nc.sync.dma_start(xT_sb[:, k, :], xT[k * 128 : (k + 1) * 128, :])
                nc.sync.dma_start(pw_sb[:, k, :], pw[k * 128 : (k + 1) * 128, :])
                nc.sync.dma_start(rw_sb[:, k, :], rw[k * 128 : (k + 1) * 128, :])
            pb_sb = cpool.tile([128, DK], F32, tag="pb")
            nc.sync.dma_start(pb_sb[:], pb[:])
            kT2_sb = cpool.tile([128, DK, E], DT1, tag="kT2")
            for k in range(DK):
                nc.sync.dma_start(kT2_sb[:, k, :], kT2[k * 128 : (k + 1) * 128, :])
            kk_sb = cpool.tile([1, E], F32, tag="kk")
            nc.sync.dma_start(kk_sb[:], kk1[:])
            onc_sb = cpool.tile([128, 1], DT1, tag="onc")
            nc.sync.dma_start(onc_sb[:], onc[:])
            onr_sb = cpool.tile([1, 512], F32, tag="onr")
            nc.sync.dma_start(onr_sb[:], onr[:])

            hT_sb = apool.tile([128, DK, PC], DT1, tag="hT")
            rT_sb = apool.tile([128, DK, PC], DT1, tag="rT")
            r2_sb = apool.tile([128, DK, PC], DT1, tag="r2")
            rr_sb = apool.tile([1, PC], F32, tag="rr")
            d2_sb = apool.tile([E, PC], F32, tag="d2")

            # h^T = pw^T-mm(x^T) + pb ; r^T = rw^T-mm(h^T)
            for w_sb, src, dst, bias in (
                (pw_sb, xT_sb, hT_sb, pb_sb),
                (rw_sb, hT_sb, rT_sb, None),
            ):
                for m in range(DK):
                    for n in range(NCH):
                        ps = pspool.tile([128, 512], F32, tag="ps")
                        for k in range(DK):
                            nc.tensor.matmul(
                                ps[:],
                                w_sb[:, k, m * 128 : (m + 1) * 128],
                                src[:, k, n * 512 : (n + 1) * 512],
                                start=(k == 0),
                                stop=(k == DK - 1),
                            )
                        if bias is not None:
                            nc.scalar.activation(
                                dst[:, m, n * 512 : (n + 1) * 512], ps[:],
                                AF.Identity, bias=bias[:, m : m + 1],
                            )
                        else:
                            nc.scalar.activation(
                                dst[:, m, n * 512 : (n + 1) * 512], ps[:], AF.Copy
                            )
            nc.sync.dma_start(hT.rearrange("(m p) n -> p m n", p=128), hT_sb[:])

            # rr = sum_d r^2 (ones-matmul over partition tiles)
            for kt in range(DK):
                nc.vector.tensor_mul(
                    r2_sb[:, kt, :], rT_sb[:, kt, :], rT_sb[:, kt, :]
                )
            for n in range(NCH):
                ps1 = psmall.tile([1, 512], F32, tag="ps1")
                for k in range(DK):
                    nc.tensor.matmul(
                        ps1[:], onc_sb[:], r2_sb[:, k, n * 512 : (n + 1) * 512],
                        start=(k == 0), stop=(k == DK - 1),
                    )
                nc.scalar.activation(rr_sb[:, n * 512 : (n + 1) * 512], ps1[:], AF.Copy)

            # d2 = (-2 keys) . r + |k|^2 + rr  (mixed fp16/fp32 psum group)
            for n in range(NCH):
                psA = psmall.tile([E, 512], F32, tag="psA")
                for k in range(DK):
                    nc.tensor.matmul(
                        psA[:], kT2_sb[:, k, :], rT_sb[:, k, n * 512 : (n + 1) * 512],
                        start=(k == 0), stop=False,
                    )
                nc.tensor.matmul(psA[:], kk_sb[:], onr_sb[:], start=False, stop=False)
                nc.tensor.matmul(
                    psA[:], onr_sb[:, 0:E], rr_sb[:, n * 512 : (n + 1) * 512],
                    start=False, stop=True,
                )
                nc.scalar.activation(d2_sb[:, n * 512 : (n + 1) * 512], psA[:], AF.Copy)
            nc.sync.dma_start(d2T[:], d2_sb[:])
    nc.compile()
    return nc


def _phase2_nc(S: int, DT2=None) -> bass.Bass:
    if DT2 is None:
        DT2 = mybir.dt.float16 if P2_F16 else F32R
    nc = bacc.Bacc("TRN2", target_bir_lowering=False, num_devices=NC)
    C = S * L
    hseg = nc.dram_tensor("hseg", [D, C], DT2, kind="ExternalInput")
    gseg = nc.dram_tensor("gseg", [128, C], F32, kind="ExternalInput")
    w1s = nc.dram_tensor("w1s", [S, D, H], DT2, kind="ExternalInput")
    w2s = nc.dram_tensor("w2s", [S, H, D], DT2, kind="ExternalInput")
    b1s = nc.dram_tensor("b1s", [128, S * HK], F32, kind="ExternalInput")
    b2s = nc.dram_tensor("b2s", [128, S * DK], F32, kind="ExternalInput")
    oseg = nc.dram_tensor("oseg", [D, C], F32, kind="ExternalOutput")

    with tile.TileContext(nc) as tc:
        with (
            tc.tile_pool(name="const", bufs=1) as cpool,
            tc.tile_pool(name="w1p", bufs=3) as w1p,
            tc.tile_pool(name="w2p", bufs=3) as w2p,
            tc.tile_pool(name="hp", bufs=3) as hp,
            tc.tile_pool(name="hidp", bufs=3) as hidp,
            tc.tile_pool(name="op", bufs=3) as op,
            tc.tile_pool(name="hid_ps", bufs=2, space="PSUM") as hidps,
            tc.tile_pool(name="out_ps", bufs=1, space="PSUM") as outps,
        ):
            gseg_sb = cpool.tile([128, C], F32, tag="gseg")
            b1_sb = cpool.tile([128, S * HK], F32, tag="b1")
            nc.sync.dma_start(b1_sb[:], b1s[:])
            b2_sb = cpool.tile([128, S * DK], F32, tag="b2")
            nc.sync.dma_start(b2_sb[:], b2s[:])

            for s in range(S):
                nc.sync.dma_start(
                    gseg_sb[:, s * L : (s + 1) * L], gseg[:, s * L : (s + 1) * L]
                )
                w1t = w1p.tile([128, DK, H], DT2, tag="w1")
                for k in range(DK):
                    nc.sync.dma_start(
                        w1t[:, k, :], w1s[s, k * 128 : (k + 1) * 128, :]
                    )
                w2t = w2p.tile([128, HK, D], DT2, tag="w2")
                for j in range(DK):
                    nc.sync.dma_start(
                        w2t[:, 4 * j : 4 * j + 4, :],
                        w2s[s, 4 * j * 128 : (4 * j + 4) * 128, :].rearrange(
                            "(k p) d -> p k d", p=128
                        ),
                    )
                ht = hp.tile([128, DK, L], DT2, tag="h")
                nc.sync.dma_start(
                    ht[:],
                    hseg[:, s * L : (s + 1) * L].rearrange("(k p) n -> p k n", p=128),
                )
                opsum = outps.tile([128, DK, L], F32, tag="opsum", name=f"opsum_{s}")
                ops = [opsum[:, mo, :] for mo in range(DK)]
                for m in range(HK):
                    hps = hidps.tile([128, L], F32, tag="hps")
                    for k in range(DK):
                        nc.tensor.matmul(
                            hps[:],
                            w1t[:, k, m * 128 : (m + 1) * 128],
                            ht[:, k, :],
                            start=(k == 0),
                            stop=(k == DK - 1),
                        )
                    hidt = hidp.tile([128, L], DT2, tag="hid")
                    nc.scalar.activation(
                        hidt[:], hps[:], AF.Gelu,
                        bias=b1_sb[:, s * HK + m : s * HK + m + 1],
                    )
                    for mo in range(DK):
                        nc.tensor.matmul(
                            ops[mo][:],
                            w2t[:, m, mo * 128 : (mo + 1) * 128],
                            hidt[:],
                            start=(m == 0),
                            stop=(m == HK - 1),
                        )
                for mo in range(DK):
                    ot = op.tile([128, L], F32, tag="o")
                    nc.vector.scalar_tensor_tensor(
                        ot[:],
                        ops[mo][:],
                        b2_sb[:, s * DK + mo : s * DK + mo + 1],
                        gseg_sb[:, s * L : (s + 1) * L],
                        ALU.add,
                        ALU.mult,
                    )
                    nc.sync.dma_start(
                        oseg[mo * 128 : (mo + 1) * 128, s * L : (s + 1) * L], ot[:]
                    )
    nc.compile()
    return nc


def _run(nc, in_maps, label):
    trace = os.environ.get("KTRACE") == "1"
    res = run_bass_kernel_spmd(
        nc, in_maps, core_ids=list(range(NC)), trace=trace
    )
    if trace:
        last_stats[label] = {
            "exec_time_ns": res.exec_time_ns,
            "mean_exec_time_ns": res.mean_exec_time_ns,
            "trace": res.instructions_and_trace[1]
            if res.instructions_and_trace
            else None,
        }
    return res.results


def kernel(view0, view1, proj_w, proj_b, router_w, expert_keys, w1, b1, w2, b2):
    view0 = np.ascontiguousarray(view0, dtype=np.float32)
    view1 = np.ascontiguousarray(view1, dtype=np.float32)
    proj_w = np.asarray(proj_w, dtype=np.float32)
    proj_b = np.asarray(proj_b, dtype=np.float32)
    router_w = np.asarray(router_w, dtype=np.float32)
    keys = np.asarray(expert_keys, dtype=np.float32)
    w1 = np.asarray(w1, dtype=np.float32)
    b1 = np.asarray(b1, dtype=np.float32)
    w2 = np.asarray(w2, dtype=np.float32)
    b2 = np.asarray(b2, dtype=np.float32)

    # ---- Phase 1: h and d2 on device (token-parallel over 8 cores) ----
    dt1 = np.float16 if P1_F16 else np.float32
    xT_full = np.concatenate(
        [view0.reshape(N, D).T, view1.reshape(N, D).T], axis=1
    )  # [D, NT], column t = view*N + (b*T + tt)
    xT_d = np.ascontiguousarray(xT_full, dtype=dt1)

    kT2 = np.ascontiguousarray(-2.0 * keys.T).astype(dt1)  # [D, E]
    kk1 = (keys * keys).sum(axis=1, dtype=np.float32).reshape(1, E)
    onc = np.ones((128, 1), dt1)
    onr = np.ones((1, 512), np.float32)

    in_maps1 = []
    for c in range(NC):
        v = (c * PC) // N  # cores 0-3 -> view 0, 4-7 -> view 1
        pb_t = np.ascontiguousarray(proj_b[v].reshape(DK, 128).T)  # [128, DK]
        in_maps1.append(
            {
                "xT": np.ascontiguousarray(xT_d[:, c * PC : (c + 1) * PC]),
                "pw": np.ascontiguousarray(proj_w[v], dtype=dt1),
                "pb": pb_t,
                "rw": np.ascontiguousarray(router_w[v], dtype=dt1),
                "kT2": kT2,
                "kk1": kk1,
                "onc": onc,
                "onr": onr,
            }
        )
    res1 = _run(_phase1_nc(), in_maps1, "phase1")

    hT_full = np.concatenate([r["hT"] for r in res1], axis=1)  # [D, NT], dt1
    d2 = np.concatenate([r["d2T"] for r in res1], axis=1).T   # [NT, E] fp32

    # ---- Host repair: recompute borderline tokens exactly in fp32 ----
    if P1_F16:
        logits0 = -np.sqrt(np.maximum(d2, 0.0), dtype=np.float32)
        part = np.partition(logits0, E - K - 1, axis=1)
        gap45 = part[:, E - K] - part[:, E - K - 1]  # 4th minus 5th logit
        risk = np.nonzero(gap45 < REPAIR_MARGIN)[0]
        last_stats["n_repaired"] = int(risk.size)
        if risk.size:
            x_all = np.concatenate(
                [view0.reshape(N, D), view1.reshape(N, D)], axis=0
            )
            vsel = (risk >= N).astype(np.int64)
            kkr = kk1.reshape(E)
            for v in (0, 1):
                rt = risk[vsel == v]
                if rt.size == 0:
                    continue
                hx = x_all[rt] @ proj_w[v] + proj_b[v]
                rx = hx @ router_w[v]
                d2[rt] = (
                    (rx * rx).sum(axis=1, keepdims=True)
                    - 2.0 * (rx @ keys.T)
                    + kkr
                )

    # ---- Host routing: logits, top-4, softmax gates (fp32) ----
    logits = -np.sqrt(np.maximum(d2, 0.0), dtype=np.float32)
    topi = np.argsort(-logits, axis=1, kind="stable")[:, :K]   # [NT, K]
    topv = np.take_along_axis(logits, topi, axis=1)
    ex = np.exp(topv - topv[:, :1], dtype=np.float32)
    gates = ex / ex.sum(axis=1, keepdims=True, dtype=np.float32)

    # ---- Slot plan: per expert, tokens cut into L-column slots ----
    slots = []  # (expert, token_ids, gate_vals)
    for e in range(E):
        sel_tok, sel_k = np.nonzero(topi == e)
        if sel_tok.size == 0:
            continue
        g_e = gates[sel_tok, sel_k]
        for i in range(0, sel_tok.size, L):
            slots.append((e, sel_tok[i : i + L], g_e[i : i + L]))
    S = max(1, math.ceil(len(slots) / NC))
    while len(slots) < S * NC:
        slots.append((-1, np.zeros(0, np.int64), np.zeros(0, np.float32)))

    # ---- Phase 2 inputs ----
    C = S * L
    dt2 = np.float16 if P2_F16 else np.float32
    w1_d = w1.astype(dt2)
    w2_d = w2.astype(dt2)
    hT_d = hT_full.astype(dt2)
    in_maps2 = []
    core_slots = []
    for c in range(NC):
        csl = slots[c * S : (c + 1) * S]
        core_slots.append(csl)
        hseg = np.zeros((D, C), dt2)
        gseg = np.zeros((1, C), np.float32)
        w1c = np.zeros((S, D, H), dt2)
        w2c = np.zeros((S, H, D), dt2)
        b1c = np.zeros((128, S * HK), np.float32)
        b2c = np.zeros((128, S * DK), np.float32)
        for s, (e, toks, gv) in enumerate(csl):
            if e < 0:
                continue
            n = toks.size
            hseg[:, s * L : s * L + n] = hT_d[:, toks]
            gseg[0, s * L : s * L + n] = gv
            w1c[s] = w1_d[e]
            w2c[s] = w2_d[e]
            b1c[:, s * HK : (s + 1) * HK] = b1[e].reshape(HK, 128).T
            b2c[:, s * DK : (s + 1) * DK] = b2[e].reshape(DK, 128).T
        in_maps2.append(
            {
                "hseg": hseg,
                "gseg": np.ascontiguousarray(
                    np.broadcast_to(gseg, (128, C))
                ),
                "w1s": w1c,
                "w2s": w2c,
                "b1s": b1c,
                "b2s": b2c,
            }
        )
    last_stats["S"] = S
    last_stats["n_slots_real"] = sum(
        1 for e, _, _ in slots if e >= 0
    )
    res2 = _run(_phase2_nc(S), in_maps2, "phase2")

    # ---- Combine ----
    fusedT = np.zeros((D, NT), np.float32)
    for c in range(NC):
        o = res2[c]["oseg"]  # [D, C]
        for s, (e, toks, _gv) in enumerate(core_slots[c]):
            if e < 0 or toks.size == 0:
                continue
            fusedT[:, toks] += o[:, s * L : s * L + toks.size]
    fused = (fusedT[:, :N] + fusedT[:, N:]).T  # [N, D]
    return np.ascontiguousarray(fused.reshape(B, T, D), dtype=np.float32)


# revision 31
# speedup vs baseline: 1.1014x; 1.1014x over previous
"""Trainium2 Bass kernel for nn_MoEElementFusion (moe_routing).

Strategy (8 NeuronCores, SPMD):
  Phase 1 (token-data-parallel): each core takes 1/8 of the 8192 (view,token)
  columns and computes, in true fp32 (PE 4-pass) to keep top-k selection
  faithful to the fp32 reference:
      h  = x @ proj_w + proj_b            (feature-major, weights stationary)
      r  = h @ router_w
      d2 = |r|^2 - 2 r.keys^T + |keys|^2  (rk/rr/kk accumulated on PE)
  Outputs h^T and d2^T.

  Host: logits = -sqrt(max(d2,0)), exact top-4 + softmax gates in numpy
  (matches the jax fp32 reference), then builds a balanced slot plan:
  each expert's selected tokens are cut into slots of L=512 columns; the
  8 cores each process S identical-length slots (expert weights + gathered
  h columns + gate rows are per-core input data, so the hot/cold expert
  imbalance is erased no matter how skewed the routing is).

  Phase 2 (compiled at runtime once S is known): per slot, FFN in fp32r
  (full PE rate at N=512):  out^T = (w2^T-mm(gelu(w1^T-mm(h^T) + b1)) + b2) * g
  feature-major throughout; gates applied via a pre-broadcast [128, C] row.

  Host combine: fused[:, tok] += out columns per slot; sum the two views.
"""

import math
import os

import numpy as np

import concourse.bass as bass
import concourse.bacc as bacc
import concourse.mybir as mybir
import concourse.tile as tile
from concourse.bass_utils import run_bass_kernel_spmd

# Problem dims (hardcoded per spec)
V, B, T, D, E, K = 2, 4, 1024, 512, 16, 4
H = 4 * D
N = B * T          # tokens per view
NT = V * N         # total (view, token) columns = 8192
NC = 8             # cores
PC = NT // NC      # phase-1 columns per core = 1024
L = 512            # phase-2 slot length (columns)

F32 = mybir.dt.float32
F32R = mybir.dt.float32r
AF = mybir.ActivationFunctionType
ALU = mybir.AluOpType

DK = D // 128      # 4 k-tiles over D
HK = H // 128      # 16 k-tiles over H

# Phase-2 FFN in fp16 (1 cycle/row on the PE vs 2 for fp32; psum stays fp32).
P2_F16 = os.environ.get("KP2F32") != "1"
# Phase-1 proj/router in fp16; borderline top-4 selections (logit gap below
# REPAIR_MARGIN) are recomputed exactly on host in fp32.
P1_F16 = os.environ.get("KP1F32") != "1"
REPAIR_MARGIN = 0.02

# Filled by kernel() for test harness introspection.
last_stats: dict = {}


def _phase1_nc() -> bass.Bass:
    DT1 = mybir.dt.float16 if P1_F16 else F32
    nc = bacc.Bacc("TRN2", target_bir_lowering=False, num_devices=NC)
    xT = nc.dram_tensor("xT", [D, PC], DT1, kind="ExternalInput")
    pw = nc.dram_tensor("pw", [D, D], DT1, kind="ExternalInput")
    pb = nc.dram_tensor("pb", [128, DK], F32, kind="ExternalInput")
    rw = nc.dram_tensor("rw", [D, D], DT1, kind="ExternalInput")
    kT2 = nc.dram_tensor("kT2", [D, E], DT1, kind="ExternalInput")
    kk1 = nc.dram_tensor("kk1", [1, E], F32, kind="ExternalInput")
    onc = nc.dram_tensor("onc", [128, 1], DT1, kind="ExternalInput")
    onr = nc.dram_tensor("onr", [1, 512], F32, kind="ExternalInput")
    hT = nc.dram_tensor("hT", [D, PC], DT1, kind="ExternalOutput")
    d2T = nc.dram_tensor("d2T", [E, PC], F32, kind="ExternalOutput")

    NCH = PC // 512  # 512-column chunks

    with tile.TileContext(nc) as tc:
        with (
            tc.tile_pool(name="const", bufs=1) as cpool,
            tc.tile_pool(name="act", bufs=1) as apool,
            tc.tile_pool(name="ps", bufs=2, space="PSUM") as pspool,
            tc.tile_pool(name="ps_small", bufs=2, space="PSUM") as psmall,
        ):
            xT_sb = cpool.tile([128, DK, PC], DT1, tag="xT")
            pw_sb = cpool.tile([128, DK, D], DT1, tag="pw")
            rw_sb = cpool.tile([128, DK, D], DT1, tag="rw")
            for k in range(DK):
                nc.sync.dma_start(xT_sb[:, k, :], xT[k * 128 : (k + 1) * 128, :])
                nc.sync.dma_start(pw_sb[:, k, :], pw[k * 128 : (k + 1) * 128, :])
                nc.sync.dma_start(rw_sb[:, k, :], rw[k * 128 : (k + 1) * 128, :])
            pb_sb = cpool.tile([128, DK], F32, tag="pb")
            nc.sync.dma_start(pb_sb[:], pb[:])
            kT2_sb = cpool.tile([128, DK, E], DT1, tag="kT2")
            for k in range(DK):
                nc.sync.dma_start(kT2_sb[:, k, :], kT2[k * 128 : (k + 1) * 128, :])
            kk_sb = cpool.tile([1, E], F32, tag="kk")
            nc.sync.dma_start(kk_sb[:], kk1[:])
            onc_sb = cpool.tile([128, 1], DT1, tag="onc")
            nc.sync.dma_start(onc_sb[:], onc[:])
            onr_sb = cpool.tile([1, 512], F32, tag="onr")
            nc.sync.dma_start(onr_sb[:], onr[:])

            hT_sb = apool.tile([128, DK, PC], DT1, tag="hT")
            rT_sb = apool.tile([128, DK, PC], DT1, tag="rT")
            r2_sb = apool.tile([128, DK, PC], DT1, tag="r2")
            rr_sb = apool.tile([1, PC], F32, tag="rr")
            d2_sb = apool.tile([E, PC], F32, tag="d2")

            # h^T = pw^T-mm(x^T) + pb ; r^T = rw^T-mm(h^T)
            for w_sb, src, dst, bias in (
                (pw_sb, xT_sb, hT_sb, pb_sb),
                (rw_sb, hT_sb, rT_sb, None),
            ):
                for m in range(DK):
                    for n in range(NCH):
                        ps = pspool.tile([128, 512], F32, tag="ps")
                        for k in range(DK):
                            nc.tensor.matmul(
                                ps[:],
                                w_sb[:, k, m * 128 : (m + 1) * 128],
                                src[:, k, n * 512 : (n + 1) * 512],
                                start=(k == 0),
                                stop=(k == DK - 1),
                            )
                        if bias is not None:
                            nc.scalar.activation(
                                dst[:, m, n * 512 : (n + 1) * 512], ps[:],
                                AF.Identity, bias=bias[:, m : m + 1],
                            )
                        else:
                            nc.scalar.activation(
                                dst[:, m, n * 512 : (n + 1) * 512], ps[:], AF.Copy
                            )
            nc.sync.dma_start(hT.rearrange("(m p) n -> p m n", p=128), hT_sb[:])

            # rr = sum_d r^2 (ones-matmul over partition tiles)
            for kt in range(DK):
                nc.vector.tensor_mul(
                    r2_sb[:, kt, :], rT_sb[:, kt, :], rT_sb[:, kt, :]
                )
            for n in range(NCH):
                ps1 = psmall.tile([1, 512], F32, tag="ps1")
                for k in range(DK):
                    nc.tensor.matmul(
                        ps1[:], onc_sb[:], r2_sb[:, k, n * 512 : (n + 1) * 512],
                        start=(k == 0), stop=(k == DK - 1),
                    )
                nc.scalar.activation(rr_sb[:, n * 512 : (n + 1) * 512], ps1[:], AF.Copy)

            # d2 = (-2 keys) . r + |k|^2 + rr  (mixed fp16/fp32 psum group)
            for n in range(NCH):
                psA = psmall.tile([E, 512], F32, tag="psA")
                for k in range(DK):
                    nc.tensor.matmul(
                        psA[:], kT2_sb[:, k, :], rT_sb[:, k, n * 512 : (n + 1) * 512],
                        start=(k == 0), stop=False,
                    )
                nc.tensor.matmul(psA[:], kk_sb[:], onr_sb[:], start=False, stop=False)
                nc.tensor.matmul(
                    psA[:], onr_sb[:, 0:E], rr_sb[:, n * 512 : (n + 1) * 512],
                    start=False, stop=True,
                )
                nc.scalar.activation(d2_sb[:, n * 512 : (n + 1) * 512], psA[:], AF.Copy)
            nc.sync.dma_start(d2T[:], d2_sb[:])
    nc.compile()
    return nc


def _phase2_nc(S: int, DT2=None) -> bass.Bass:
    if DT2 is None:
        DT2 = mybir.dt.float16 if P2_F16 else F32R
    nc = bacc.Bacc("TRN2", target_bir_lowering=False, num_devices=NC)
    C = S * L
    hseg = nc.dram_tensor("hseg", [D, C], DT2, kind="ExternalInput")
    gseg = nc.dram_tensor("gseg", [128, C], F32, kind="ExternalInput")
    w1s = nc.dram_tensor("w1s", [S, D, H], DT2, kind="ExternalInput")
    w2s = nc.dram_tensor("w2s", [S, H, D], DT2, kind="ExternalInput")
    b1s = nc.dram_tensor("b1s", [128, S * HK], F32, kind="ExternalInput")
    b2s = nc.dram_tensor("b2s", [128, S * DK], F32, kind="ExternalInput")
    oseg = nc.dram_tensor("oseg", [D, C], F32, kind="ExternalOutput")

    with tile.TileContext(nc) as tc:
        with (
            tc.tile_pool(name="const", bufs=1) as cpool,
            tc.tile_pool(name="w1p", bufs=3) as w1p,
            tc.tile_pool(name="w2p", bufs=3) as w2p,
            tc.tile_pool(name="hp", bufs=3) as hp,
            tc.tile_pool(name="hidp", bufs=3) as hidp,
            tc.tile_pool(name="op", bufs=3) as op,
            tc.tile_pool(name="hid_ps", bufs=2, space="PSUM") as hidps,
            tc.tile_pool(name="out_ps", bufs=1, space="PSUM") as outps,
        ):
            gseg_sb = cpool.tile([128, C], F32, tag="gseg")
            b1_sb = cpool.tile([128, S * HK], F32, tag="b1")
            nc.sync.dma_start(b1_sb[:], b1s[:])
            b2_sb = cpool.tile([128, S * DK], F32, tag="b2")
            nc.sync.dma_start(b2_sb[:], b2s[:])

            for s in range(S):
                ht = hp.tile([128, DK, L], DT2, tag="h")
                for k in range(DK):
                    nc.sync.dma_start(
                        ht[:, k, :],
                        hseg[k * 128 : (k + 1) * 128, s * L : (s + 1) * L],
                    )
                w1t = w1p.tile([128, DK, H], DT2, tag="w1")
                for k in range(DK):
                    nc.sync.dma_start(
                        w1t[:, k, :], w1s[s, k * 128 : (k + 1) * 128, :]
                    )
                w2t = w2p.tile([128, HK, D], DT2, tag="w2")
                for j in range(DK):
                    nc.sync.dma_start(
                        w2t[:, 4 * j : 4 * j + 4, :],
                        w2s[s, 4 * j * 128 : (4 * j + 4) * 128, :].rearrange(
                            "(k p) d -> p k d", p=128
                        ),
                    )
                nc.sync.dma_start(
                    gseg_sb[:, s * L : (s + 1) * L], gseg[:, s * L : (s + 1) * L]
                )
                opsum = outps.tile([128, DK, L], F32, tag="opsum", name=f"opsum_{s}")
                ops = [opsum[:, mo, :] for mo in range(DK)]
                for m in range(HK):
                    hps = hidps.tile([128, L], F32, tag="hps")
                    for k in range(DK):
                        nc.tensor.matmul(
                            hps[:],
                            w1t[:, k, m * 128 : (m + 1) * 128],
                            ht[:, k, :],
                            start=(k == 0),
                            stop=(k == DK - 1),
                        )
                    hidt = hidp.tile([128, L], DT2, tag="hid")
                    nc.scalar.activation(
                        hidt[:], hps[:], AF.Gelu,
                        bias=b1_sb[:, s * HK + m : s * HK + m + 1],
                    )
                    for mo in range(DK):
                        nc.tensor.matmul(
                            ops[mo][:],
                            w2t[:, m, mo * 128 : (mo + 1) * 128],
                            hidt[:],
                            start=(m == 0),
                            stop=(m == HK - 1),
                        )
                for mo in range(DK):
                    ot = op.tile([128, L], F32, tag="o")
                    nc.vector.scalar_tensor_tensor(
                        ot[:],
                        ops[mo][:],
                        b2_sb[:, s * DK + mo : s * DK + mo + 1],
                        gseg_sb[:, s * L : (s + 1) * L],
                        ALU.add,
                        ALU.mult,
                    )
                    nc.sync.dma_start(
                        oseg[mo * 128 : (mo + 1) * 128, s * L : (s + 1) * L], ot[:]
                    )
    nc.compile()
    return nc


def _run(nc, in_maps, label):
    trace = os.environ.get("KTRACE") == "1"
    res = run_bass_kernel_spmd(
        nc, in_maps, core_ids=list(range(NC)), trace=trace
    )
    if trace:
        last_stats[label] = {
            "exec_time_ns": res.exec_time_ns,
            "mean_exec_time_ns": res.mean_exec_time_ns,
            "trace": res.instructions_and_trace[1]
            if res.instructions_and_trace
            else None,
        }
    return res.results


def kernel(view0, view1, proj_w, proj_b, router_w, expert_keys, w1, b1, w2, b2):
    view0 = np.ascontiguousarray(view0, dtype=np.float32)
    view1 = np.ascontiguousarray(view1, dtype=np.float32)
    proj_w = np.asarray(proj_w, dtype=np.float32)
    proj_b = np.asarray(proj_b, dtype=np.float32)
    router_w = np.asarray(router_w, dtype=np.float32)
    keys = np.asarray(expert_keys, dtype=np.float32)
    w1 = np.asarray(w1, dtype=np.float32)
    b1 = np.asarray(b1, dtype=np.float32)
    w2 = np.asarray(w2, dtype=np.float32)
    b2 = np.asarray(b2, dtype=np.float32)

    # ---- Phase 1: h and d2 on device (token-parallel over 8 cores) ----
    dt1 = np.float16 if P1_F16 else np.float32
    xT_full = np.concatenate(
        [view0.reshape(N, D).T, view1.reshape(N, D).T], axis=1
    )  # [D, NT], column t = view*N + (b*T + tt)
    xT_d = np.ascontiguousarray(xT_full, dtype=dt1)

    kT2 = np.ascontiguousarray(-2.0 * keys.T).astype(dt1)  # [D, E]
    kk1 = (keys * keys).sum(axis=1, dtype=np.float32).reshape(1, E)
    onc = np.ones((128, 1), dt1)
    onr = np.ones((1, 512), np.float32)

    in_maps1 = []
    for c in range(NC):
        v = (c * PC) // N  # cores 0-3 -> view 0, 4-7 -> view 1
        pb_t = np.ascontiguousarray(proj_b[v].reshape(DK, 128).T)  # [128, DK]
        in_maps1.append(
            {
                "xT": np.ascontiguousarray(xT_d[:, c * PC : (c + 1) * PC]),
                "pw": np.ascontiguousarray(proj_w[v], dtype=dt1),
                "pb": pb_t,
                "rw": np.ascontiguousarray(router_w[v], dtype=dt1),
                "kT2": kT2,
                "kk1": kk1,
                "onc": onc,
                "onr": onr,
            }
        )
    res1 = _run(_phase1_nc(), in_maps1, "phase1")

    hT_full = np.concatenate([r["hT"] for r in res1], axis=1)  # [D, NT], dt1
    d2 = np.concatenate([r["d2T"] for r in res1], axis=1).T   # [NT, E] fp32

    # ---- Host repair: recompute borderline tokens exactly in fp32 ----
    if P1_F16:
        logits0 = -np.sqrt(np.maximum(d2, 0.0), dtype=np.float32)
        part = np.partition(logits0, E - K - 1, axis=1)
        gap45 = part[:, E - K] - part[:, E - K - 1]  # 4th minus 5th logit
        risk = np.nonzero(gap45 < REPAIR_MARGIN)[0]
        last_stats["n_repaired"] = int(risk.size)
        if risk.size:
            x_all = np.concatenate(
                [view0.reshape(N, D), view1.reshape(N, D)], axis=0
            )
            vsel = (risk >= N).astype(np.int64)
            kkr = kk1.reshape(E)
            for v in (0, 1):
                rt = risk[vsel == v]
                if rt.size == 0:
                    continue
                hx = x_all[rt] @ proj_w[v] + proj_b[v]
                rx = hx @ router_w[v]
                d2[rt] = (
                    (rx * rx).sum(axis=1, keepdims=True)
                    - 2.0 * (rx @ keys.T)
                    + kkr
                )

    # ---- Host routing: logits, top-4, softmax gates (fp32) ----
    logits = -np.sqrt(np.maximum(d2, 0.0), dtype=np.float32)
    topi = np.argsort(-logits, axis=1, kind="stable")[:, :K]   # [NT, K]
    topv = np.take_along_axis(logits, topi, axis=1)
    ex = np.exp(topv - topv[:, :1], dtype=np.float32)
    gates = ex / ex.sum(axis=1, keepdims=True, dtype=np.float32)

    # ---- Slot plan: per expert, tokens cut into L-column slots ----
    slots = []  # (expert, token_ids, gate_vals)
    for e in range(E):
        sel_tok, sel_k = np.nonzero(topi == e)
        if sel_tok.size == 0:
            continue
        g_e = gates[sel_tok, sel_k]
        for i in range(0, sel_tok.size, L):
            slots.append((e, sel_tok[i : i + L], g_e[i : i + L]))
    S = max(1, math.ceil(len(slots) / NC))
    while len(slots) < S * NC:
        slots.append((-1, np.zeros(0, np.int64), np.zeros(0, np.float32)))

    # ---- Phase 2 inputs ----
    C = S * L
    dt2 = np.float16 if P2_F16 else np.float32
    w1_d = w1.astype(dt2)
    w2_d = w2.astype(dt2)
    hT_d = hT_full.astype(dt2)
    in_maps2 = []
    core_slots = []
    for c in range(NC):
        csl = slots[c * S : (c + 1) * S]
        core_slots.append(csl)
        hseg = np.zeros((D, C), dt2)
        gseg = np.zeros((1, C), np.float32)
        w1c = np.zeros((S, D, H), dt2)
        w2c = np.zeros((S, H, D), dt2)
        b1c = np.zeros((128, S * HK), np.float32)
        b2c = np.zeros((128, S * DK), np.float32)
        for s, (e, toks, gv) in enumerate(csl):
            if e < 0:
                continue
            n = toks.size
            hseg[:, s * L : s * L + n] = hT_d[:, toks]
            gseg[0, s * L : s * L + n] = gv
            w1c[s] = w1_d[e]
            w2c[s] = w2_d[e]
            b1c[:, s * HK : (s + 1) * HK] = b1[e].reshape(HK, 128).T
            b2c[:, s * DK : (s + 1) * DK] = b2[e].reshape(DK, 128).T
        in_maps2.append(
            {
                "hseg": hseg,
                "gseg": np.ascontiguousarray(
                    np.broadcast_to(gseg, (128, C))
                ),
                "w1s": w1c,
                "w2s": w2c,
                "b1s": b1c,
                "b2s": b2c,
            }
        )
    last_stats["S"] = S
    last_stats["n_slots_real"] = sum(
        1 for e, _, _ in slots if e >= 0
    )
    res2 = _run(_phase2_nc(S), in_maps2, "phase2")

    # ---- Combine ----
    fusedT = np.zeros((D, NT), np.float32)
    for c in range(NC):
        o = res2[c]["oseg"]  # [D, C]
        for s, (e, toks, _gv) in enumerate(core_slots[c]):
            if e < 0 or toks.size == 0:
                continue
            fusedT[:, toks] += o[:, s * L : s * L + toks.size]
    fused = (fusedT[:, :N] + fusedT[:, N:]).T  # [N, D]
    return np.ascontiguousarray(fused.reshape(B, T, D), dtype=np.float32)
